# revision 33
# baseline (speedup 1.0000x reference)
"""Trainium2 Bass kernel for a masked-attention block (MAB).

Computation (per batch element):
    Q = X@Wq + bq ; K = Y@Wk + bk ; V = Y@Wv + bv
    logits = per-head Qh@Kh^T / 32, masked keys -> -inf, softmax over keys
    attn   = A @ Vh (concat heads)
    O1 = LN(Q + attn; g1,b1)
    O  = LN(O1 + relu(O1@Wo + bo); g2,b2)

Sharding: pure data-parallel, one batch element per NeuronCore (B=8 = 8 cores).

On-device dataflow is "feature-major": activations live in SBUF transposed
([model_dim -> 8x128 partitions, token -> free]).  With weights in natural
layout every matmul chains without any transposes.

Precision plan (correctness gate is rel_err < 2e-2, leaving large headroom):
  * Q-proj / logits / O-proj in bf16 (these touch the residual path directly,
    bf16 keeps their error ~0.5%).  bf16 moving operands are 1024 wide, which
    halves the matmul dispatch count vs fp32r's 512 limit.
  * K-proj / V-proj / softmax-exp / denominator / AV in fp8e4 with
    MatmulPerfMode.DoubleRow (2 contraction sub-tiles per pass = 2x MAC rate).
    All of these errors reach the output diluted ~30x: attention output is a
    ~900-key weighted mean added to the much larger Q residual, and the
    denominator/numerator share the same quantized exp so the softmax
    normalization error largely cancels.
  * LayerNorm stats via bf16 all-ones matmuls on bf16 activations
    (partition-dim reduction + broadcast in one shot).
  * V bias is folded into the attention epilogue (softmax weights sum to 1,
    so  sum_k A_k (V_k + bv) = pa*rc + bv), saving the bias-fold matmuls.

The host converts X->bf16, Y->fp8, Wq/Wo->bf16, Wk/Wv->fp8 and transposes
X/Y, which also cuts input HBM traffic from 28MB to ~9MB per core.
"""

import math
import numpy as np
from contextlib import ExitStack

import ml_dtypes

import concourse.bass as bass
import concourse.mybir as mybir
import concourse.tile as tile
from concourse import bacc
from concourse.bass_utils import run_bass_kernel_spmd

P = 128
NX = 1024
NY = 1024
DIM = 1024
H = 8
KO = DIM // P          # 8 partition sub-tiles of the model dim
QC = 512               # half-chunk of the query dim (for LN/O-proj pipelining)
NQC = NX // QC         # 2
F32 = mybir.dt.float32
BF16 = mybir.dt.bfloat16
F8 = mybir.dt.float8e4
AF = mybir.ActivationFunctionType
ALU = mybir.AluOpType
DR = mybir.MatmulPerfMode.DoubleRow
SCALE = 1.0 / 32.0     # 1/sqrt(DIM)
EPS = 1e-5
# walrus ships --enable-ldw-opt=false; flipping it to true dies in walrus
# codegen (visitInstLdweights INTERNAL_ERROR) with this kernel's DoubleRow
# matmuls, so it stays off.
ENABLE_LDW_OPT = False

_LDW_PATCHED = False


def _patch_ldw_opt():
    global _LDW_PATCHED
    if _LDW_PATCHED or not ENABLE_LDW_OPT:
        return
    import concourse.bass_utils as _bu
    _orig = _bu.run_command

    def _run_command(argv, **kwargs):
        argv = ["--enable-ldw-opt=true" if a == "--enable-ldw-opt=false" else a
                for a in argv]
        return _orig(argv, **kwargs)

    _bu.run_command = _run_command
    _LDW_PATCHED = True


def _build():
    _patch_ldw_opt()
    nc = bacc.Bacc("TRN2", target_bir_lowering=False, debug=False,
                   enable_asserts=False)

    # ---- DRAM I/O (per-core shapes) ----
    XT = nc.dram_tensor("XT", [DIM, NX], BF16, kind="ExternalInput").ap()
    YT8 = nc.dram_tensor("YT8", [DIM, NY], F8, kind="ExternalInput").ap()
    MB = nc.dram_tensor("MB", [NY], F32, kind="ExternalInput").ap()
    Wq = nc.dram_tensor("Wq", [DIM, DIM], BF16, kind="ExternalInput").ap()
    Wk = nc.dram_tensor("Wk", [DIM, DIM], F8, kind="ExternalInput").ap()
    Wv = nc.dram_tensor("Wv", [DIM, DIM], F8, kind="ExternalInput").ap()
    Wo = nc.dram_tensor("Wo", [DIM, DIM], BF16, kind="ExternalInput").ap()
    Vecs = {}
    for vname in ("bq", "bk", "bv", "bo", "g1", "b1", "g2", "b2"):
        Vecs[vname] = nc.dram_tensor(vname, [DIM], F32, kind="ExternalInput").ap()
    OT = nc.dram_tensor("OT", [DIM, NX], BF16, kind="ExternalOutput").ap()

    xt3 = XT.rearrange("(ko p) q -> p ko q", p=P)
    yt3 = YT8.rearrange("(ko p) q -> p ko q", p=P)
    wq3 = Wq.rearrange("(ko p) d -> p ko d", p=P)
    wk3 = Wk.rearrange("(ko p) d -> p ko d", p=P)
    wv3 = Wv.rearrange("(ko p) d -> p ko d", p=P)
    wo3 = Wo.rearrange("(ko p) d -> p ko d", p=P)
    ot3 = OT.rearrange("(do p) q -> p do q", p=P)

    with tile.TileContext(nc) as tc:
        with ExitStack() as octx:
            const = octx.enter_context(tc.tile_pool(name="const", bufs=1))
            # big activation tiles; o1t reuses qt's slot, z2t reuses zt's
            # (same tag + bufs=1 -> the tile framework rotates in place and
            # carries the WAR dependency for us)
            bigp = octx.enter_context(tc.tile_pool(name="big", bufs=1))

            # ---- constants ----
            ones_bf = const.tile([P, P], BF16, tag="onesbf", name="ones_bf")
            nc.vector.memset(ones_bf, 1.0)
            ones8 = const.tile([P, 2, P], F8, tag="ones8", name="ones8")
            nc.vector.memset(ones8, 1.0)
            eps_sb = const.tile([P, 1], F32, tag="eps", name="eps_sb")
            nc.vector.memset(eps_sb, EPS)

            # vec tiles are allocated here but their (scattered, slow) DMAs
            # are issued after the big input DMAs so they don't delay the
            # first matmuls
            def vec_pko(name):
                return const.tile([P, KO], F32, tag=f"v_{name}", name=f"{name}_sb")

            mb_sb = const.tile([P, KO], F32, tag="v_mb", name="mb_sb")
            bq_sb = vec_pko("bq")
            bk_sb = vec_pko("bk")
            bv_sb = vec_pko("bv")
            bo_sb = vec_pko("bo")
            g1_sb = vec_pko("g1")
            b1_sb = vec_pko("b1")
            g2_sb = vec_pko("g2")
            b2_sb = vec_pko("b2")

            def issue_vec_dmas():
                nc.sync.dma_start(mb_sb, MB.rearrange("(ko p) -> p ko", p=P))
                for name, t in (("bq", bq_sb), ("bk", bk_sb), ("bv", bv_sb),
                                ("bo", bo_sb), ("g1", g1_sb), ("b1", b1_sb),
                                ("g2", g2_sb), ("b2", b2_sb)):
                    nc.sync.dma_start(t, Vecs[name].rearrange("(ko p) -> p ko", p=P))

            # ---- big feature-major activation tiles ----
            qt = bigp.tile([P, KO, NX], BF16, tag="bigA", name="qt")
            ktm = bigp.tile([P, KO, NY], BF16, tag="bigK", name="ktm")
            vm8 = bigp.tile([P, KO, DIM], F8, tag="bigV", name="vm8")
            zt = bigp.tile([P, KO, NX], BF16, tag="bigB", name="zt")
            wo_sb = bigp.tile([P, KO, DIM], BF16, tag="bigW", name="wo_sb")

            # ============ Phase 1: projections (+ attention) ============
            with tc.tile_pool(name="io", bufs=1) as iop, \
                 tc.tile_pool(name="w1", bufs=1) as wp:
                yt8 = iop.tile([P, KO, NY], F8, tag="yt", name="yt8")
                xt = iop.tile([P, KO, NX], BF16, tag="xt", name="xt")
                wk8 = wp.tile([P, KO, DIM], F8, tag="wk", name="wk8")
                wq_sb = wp.tile([P, KO, DIM], BF16, tag="wq", name="wq_sb")
                wv8 = wp.tile([P, KO, DIM], F8, tag="wv", name="wv8")
                # DMA order == need order: K-proj inputs, Q-proj inputs,
                # V weights, O weights (arrives during attention)
                # interleave Y/Wk so the first K-proj k-pair (4 DMAs) lands
                # as early as possible -- the accumulation then streams
                # behind the DMA queue instead of waiting for all 16
                for k in range(KO):
                    nc.sync.dma_start(yt8[:, k, :], yt3[:, k, :])
                    nc.sync.dma_start(wk8[:, k, :], wk3[:, k, :])
                issue_vec_dmas()
                for k in range(KO):
                    nc.sync.dma_start(xt[:, k, :], xt3[:, k, :])
                for k in range(KO):
                    nc.sync.dma_start(wq_sb[:, k, :], wq3[:, k, :])
                for k in range(KO):
                    nc.sync.dma_start(wv8[:, k, :], wv3[:, k, :])
                for k in range(KO):
                    nc.sync.dma_start(wo_sb[:, k, :], wo3[:, k, :])

                with tc.tile_pool(name="kqp", bufs=2, space="PSUM") as kqp:
                    # ~36 dummy matmuls on constants while the first input
                    # DMAs stream in: keeps the PE busy so the HAM clock
                    # gate is at 8/8 (2.4 GHz) when the real matmuls start,
                    # instead of paying the cold 1.2 GHz ramp on them
                    warm_rhs = const.tile([P, QC], BF16, tag="warm",
                                          name="warm_rhs")
                    nc.vector.memset(warm_rhs, 0.0)
                    warm_ps = kqp.tile([P, QC], F32, tag="warmps",
                                       name="warm_ps")
                    for _ in range(36):
                        nc.tensor.matmul(warm_ps, lhsT=ones_bf, rhs=warm_rhs,
                                         start=True, stop=True)

                    # K-proj: fp8 DoubleRow, feature-major out -> ktm bf16
                    for no in range(KO):
                        for qc in range(NQC):
                            qs = slice(qc * QC, (qc + 1) * QC)
                            ps = kqp.tile([P, QC], F32, tag="kps",
                                          name=f"kps{no}{qc}")
                            for kp in range(KO // 2):
                                nc.tensor.matmul(
                                    ps,
                                    lhsT=wk8[:, 2 * kp:2 * kp + 2,
                                             no * P:(no + 1) * P],
                                    rhs=yt8[:, 2 * kp:2 * kp + 2, qs],
                                    start=(kp == 0), stop=(kp == KO // 2 - 1),
                                    perf_mode=DR)
                            nc.scalar.activation(
                                ktm[:, no, qs], ps, AF.Identity,
                                bias=bk_sb[:, no:no + 1], scale=1.0)

                    # Q-proj: bf16 -> qt bf16 (psum writes are 512-wide: a
                    # matmul output cannot cross a PSUM bank; the ACT reads
                    # the whole 2-bank tile in one op)
                    for no in range(KO):
                        psq = kqp.tile([P, NX], F32, tag="qps", name=f"qps{no}")
                        for k in range(KO):
                            for qc in range(NQC):
                                qs = slice(qc * QC, (qc + 1) * QC)
                                nc.tensor.matmul(
                                    psq[:, qs],
                                    lhsT=wq_sb[:, k, no * P:(no + 1) * P],
                                    rhs=xt[:, k, qs],
                                    start=(k == 0), stop=(k == KO - 1))
                        nc.scalar.activation(
                            qt[:, no, :], psq, AF.Identity,
                            bias=bq_sb[:, no:no + 1], scale=1.0)

                # ============ Phase 2: attention ============
                # V-proj matmuls are interleaved into the first heads: the
                # attention phase is ACT(exp)-bound, so the PE has idle slots.
                with tc.tile_pool(name="lgp", bufs=2, space="PSUM") as lgp, \
                     tc.tile_pool(name="vpp", bufs=2, space="PSUM") as vpp, \
                     tc.tile_pool(name="prp", bufs=1, space="PSUM") as prp, \
                     tc.tile_pool(name="pap", bufs=1, space="PSUM") as pap, \
                     tc.tile_pool(name="etp", bufs=2) as etp, \
                     tc.tile_pool(name="rcp", bufs=2) as rcp, \
                     tc.tile_pool(name="ept", bufs=3) as ept:

                    def vproj(ng):
                        # V in natural (token-major) layout, fp8 DoubleRow:
                        # V[y, n] = sum_k Y[y, k] Wv[k, n]
                        ns = slice(ng * QC, (ng + 1) * QC)
                        for yo in range(KO):
                            ps = vpp.tile([P, QC], F32, tag="vps",
                                          name=f"vps{ng}{yo}")
                            for kp in range(KO // 2):
                                nc.tensor.matmul(
                                    ps,
                                    lhsT=yt8[:, 2 * kp:2 * kp + 2,
                                             yo * P:(yo + 1) * P],
                                    rhs=wv8[:, 2 * kp:2 * kp + 2, ns],
                                    start=(kp == 0), stop=(kp == KO // 2 - 1),
                                    perf_mode=DR)
                            # no bias here: bv is folded into the epilogue
                            nc.vector.tensor_copy(vm8[:, yo, ns], ps)

                    def logits_exp(h):
                        # logitsT[k, q] = sum_d KT_h[d, k] QT_h[d, q]; exp with
                        # mask bias per key (partition) and 1/32 scale, output
                        # straight to fp8 (feeds only the fp8 DR matmuls).
                        et8 = etp.tile([P, KO, NY], F8, tag="et", name=f"et{h}")
                        for kt in range(KO):
                            pl = lgp.tile([P, NX], F32, tag="lg",
                                          name=f"pl{h}{kt}")
                            for qc in range(NQC):
                                qs = slice(qc * QC, (qc + 1) * QC)
                                nc.tensor.matmul(
                                    pl[:, qs],
                                    lhsT=ktm[:, h, kt * P:(kt + 1) * P],
                                    rhs=qt[:, h, qs],
                                    start=True, stop=True)
                            nc.scalar.activation(
                                et8[:, kt, :], pl, AF.Exp,
                                bias=mb_sb[:, kt:kt + 1], scale=SCALE)
                        return et8

                    def denom_av(h, et8):
                        # softmax denominator via all-ones fp8 DR matmul
                        # (partition reduction AND broadcast in one shot),
                        # then attnT_h = V^T expT, normalized + bv + Q resid.
                        rc = rcp.tile([P, NX], F32, tag="rc", name=f"rc{h}")
                        for qc in range(NQC):
                            qs = slice(qc * QC, (qc + 1) * QC)
                            pr = prp.tile([P, QC], F32, tag="pr",
                                          name=f"pr{h}{qc}")
                            for kp in range(KO // 2):
                                nc.tensor.matmul(
                                    pr, lhsT=ones8,
                                    rhs=et8[:, 2 * kp:2 * kp + 2, qs],
                                    start=(kp == 0), stop=(kp == KO // 2 - 1),
                                    perf_mode=DR)
                            nc.vector.reciprocal_approx_fast(rc[:, qs], pr)
                        for qc in range(NQC):
                            qs = slice(qc * QC, (qc + 1) * QC)
                            pa = pap.tile([P, QC], F32, tag="pa",
                                          name=f"pa{h}{qc}")
                            for kp in range(KO // 2):
                                nc.tensor.matmul(
                                    pa,
                                    lhsT=vm8[:, 2 * kp:2 * kp + 2,
                                             h * P:(h + 1) * P],
                                    rhs=et8[:, 2 * kp:2 * kp + 2, qs],
                                    start=(kp == 0), stop=(kp == KO // 2 - 1),
                                    perf_mode=DR)
                            t1 = ept.tile([P, QC], BF16, tag="t1",
                                          name=f"t1{h}{qc}")
                            nc.vector.tensor_mul(t1, pa, rc[:, qs])
                            # zt = (pa*rc + bv) + qt   (bv fold: sum_k A_k = 1)
                            nc.vector.scalar_tensor_tensor(
                                zt[:, h, qs], t1, bv_sb[:, h:h + 1],
                                qt[:, h, qs], op0=ALU.add, op1=ALU.add)

                    # software pipeline: head h's logits/exp (PE+ACT) run while
                    # head h-1's denominator+AV (PE) wait on h-1's exp
                    prev = None
                    for h in range(H):
                        et8 = logits_exp(h)
                        if h == 0:
                            vproj(0)
                        elif h == 1:
                            vproj(1)
                        if prev is not None:
                            denom_av(h - 1, prev)
                        prev = et8
                    denom_av(H - 1, prev)

            # ---- LayerNorm over the model dim (partition direction) ----
            # engine split: squares + final scale/shift on ACT, stats chain +
            # normalize on DVE, reductions on PE (the back half is otherwise
            # DVE-serialized)
            def layernorm(in_sb, q0, QL, sqp, stp, spp, emit_out,
                          use_gpsimd=True):
                qc = q0 // QL
                qs = slice(q0, q0 + QL)
                # stats psums allocated at full chunk size and sliced, so
                # 256-wide chunks don't cost extra PSUM bank tags
                pmu = spp.tile([P, QC], F32, tag="pmu", name=f"pmu{qc}")[:, :QL]
                ps2 = spp.tile([P, QC], F32, tag="ps2", name=f"ps2{qc}")[:, :QL]
                for do in range(KO):
                    nc.tensor.matmul(pmu, lhsT=ones_bf,
                                     rhs=in_sb[:, do, qs],
                                     start=(do == 0), stop=(do == KO - 1))
                # squared sums: ACT writes fp8 pairs, reduced with a
                # DoubleRow all-ones matmul (half the matmul count; the fp8
                # quantization of x^2 averages out over the 1024 features)
                for dp in range(KO // 2):
                    sq = sqp.tile([P, 2, QL], F8, tag=f"sq{QL}", name=f"sq{qc}{dp}")
                    nc.scalar.square(sq[:, 0, :], in_sb[:, 2 * dp, qs])
                    nc.scalar.square(sq[:, 1, :], in_sb[:, 2 * dp + 1, qs])
                    nc.tensor.matmul(ps2, lhsT=ones8, rhs=sq,
                                     start=(dp == 0), stop=(dp == KO // 2 - 1),
                                     perf_mode=DR)
                mu = stp.tile([P, QL], F32, tag=f"mu{QL}", name=f"mu{qc}")
                nc.vector.tensor_scalar_mul(mu, pmu, 1.0 / DIM)
                msq = stp.tile([P, QL], F32, tag=f"msq{QL}", name=f"msq{qc}")
                nc.vector.tensor_mul(msq, mu, mu)
                sd = stp.tile([P, QL], F32, tag=f"sd{QL}", name=f"sd{qc}")
                nc.vector.scalar_tensor_tensor(
                    sd, ps2, 1.0 / DIM, msq,
                    op0=ALU.mult, op1=ALU.subtract)
                nc.scalar.activation(sd, sd, AF.Sqrt, bias=eps_sb, scale=1.0)
                # bf16 rsig/mrs/t: both normalize tensor-tensor ops then run
                # in the DVE 2x_1P packed mode (16-bit, unit stride); the
                # last two feature blocks go to the otherwise-idle GpSimd
                rsig32 = stp.tile([P, QL], F32, tag=f"rsig32{QL}", name=f"rsig32{qc}")
                nc.vector.reciprocal_approx_fast(rsig32, sd)
                rsig = stp.tile([P, QL], BF16, tag=f"rsig{QL}", name=f"rsig{qc}")
                nc.vector.tensor_copy(rsig, rsig32)
                mrs = stp.tile([P, QL], BF16, tag=f"mrs{QL}", name=f"mrs{qc}")
                nc.vector.tensor_mul(mrs, mu, rsig32)
                for do in range(KO):
                    # GpSimd runs ~4x slower than packed DVE: give it one
                    # block mid-kernel, none in LN2 where its latency would
                    # sit on the critical end-of-kernel path
                    eng = nc.gpsimd if (use_gpsimd and do == KO - 1) else nc.vector
                    t = sqp.tile([P, QL], BF16, tag=f"t{QL}", name=f"t{qc}{do}")
                    eng.tensor_mul(t, in_sb[:, do, qs], rsig)
                    eng.tensor_sub(t, t, mrs)
                    emit_out(do, qs, t)

            # ============ Phase 3: LN1 / O-proj / LN2, pipelined by
            # query half-chunk so LN's DVE latency hides under O-proj ====
            o1t = bigp.tile([P, KO, NX], BF16, tag="bigA", name="o1t")
            z2t = bigp.tile([P, KO, NX], BF16, tag="bigB", name="z2t")

            def emit_o1(do, qs, t):
                # o1 = t*g1 + b1 on ACT (activation computes f(in*scale+bias))
                nc.scalar.activation(
                    o1t[:, do, qs], t, AF.Identity,
                    bias=b1_sb[:, do:do + 1], scale=g1_sb[:, do:do + 1])

            with tc.tile_pool(name="sq1", bufs=3) as sqp1, \
                 tc.tile_pool(name="st1", bufs=2) as stp1, \
                 tc.tile_pool(name="sq2", bufs=4) as sqp2, \
                 tc.tile_pool(name="st2", bufs=2) as stp2, \
                 tc.tile_pool(name="out", bufs=4) as outp, \
                 tc.tile_pool(name="sp1", bufs=1, space="PSUM") as spp1, \
                 tc.tile_pool(name="gp3", bufs=4, space="PSUM") as pp3, \
                 tc.tile_pool(name="sp2", bufs=1, space="PSUM") as spp2:

                def oproj(q0, ql):
                    # HT[n, q] = sum_d Wo[d, n] O1T[d, q]; z2 = o1 + relu(H+bo)
                    # NOTE: keep no-outer/k-inner -- a k-outer variant that
                    # cycles psum banks every matmul measured 15-20% slower
                    # (HAM psum-queue depth-cycling)
                    qs = slice(q0, q0 + ql)
                    qc = q0 // 256
                    for no in range(KO):
                        ps = pp3.tile([P, QC], F32, tag="ps",
                                      name=f"ps_o{qc}{no}")[:, :ql]
                        for k in range(KO):
                            nc.tensor.matmul(
                                ps,
                                lhsT=wo_sb[:, k, no * P:(no + 1) * P],
                                rhs=o1t[:, k, qs],
                                start=(k == 0), stop=(k == KO - 1))
                        ht = sqp2.tile([P, QC], BF16, tag="ht",
                                       name=f"ht{qc}{no}")[:, :ql]
                        nc.scalar.activation(ht, ps, AF.Relu,
                                             bias=bo_sb[:, no:no + 1], scale=1.0)
                        # all-bf16 add -> DVE 2x packed mode
                        nc.vector.tensor_add(z2t[:, no, qs], ht, o1t[:, no, qs])

                def emit_o2(do, qs, t):
                    ql = qs.stop - qs.start
                    o = outp.tile([P, ql], BF16, tag=f"o{ql}", name=f"o{do}")
                    nc.scalar.activation(
                        o, t, AF.Identity,
                        bias=b2_sb[:, do:do + 1], scale=g2_sb[:, do:do + 1])
                    nc.sync.dma_start(ot3[:, do, qs], o)

                layernorm(zt, 0, QC, sqp1, stp1, spp1, emit_o1)
                oproj(0, QC)
                layernorm(zt, QC, QC, sqp1, stp1, spp1, emit_o1)
                oproj(QC, QC)
                layernorm(z2t, 0, QC, sqp2, stp2, spp2, emit_o2,
                          use_gpsimd=False)
                layernorm(z2t, QC, QC, sqp2, stp2, spp2, emit_o2,
                          use_gpsimd=False)

    nc.compile()
    return nc


_CACHE = {}


def _get_nc():
    if "nc" not in _CACHE:
        _CACHE["nc"] = _build()
    return _CACHE["nc"]


def make_in_maps(X, Y, mask, Wq, bq, Wk, bk, Wv, bv, Wo, bo, g1, b1, g2, b2):
    f32 = lambda a: np.ascontiguousarray(np.asarray(a, dtype=np.float32))
    bf = lambda a: np.ascontiguousarray(
        np.asarray(a, dtype=np.float32).astype(ml_dtypes.bfloat16))
    f8 = lambda a: np.ascontiguousarray(
        np.asarray(a, dtype=np.float32).astype(ml_dtypes.float8_e4m3))
    shared = {
        "Wq": bf(Wq), "Wk": f8(Wk), "Wv": f8(Wv), "Wo": bf(Wo),
        "bq": f32(bq), "bk": f32(bk), "bv": f32(bv), "bo": f32(bo),
        "g1": f32(g1), "b1": f32(b1), "g2": f32(g2), "b2": f32(b2),
    }
    X = np.asarray(X, dtype=np.float32)
    Y = np.asarray(Y, dtype=np.float32)
    mask = np.asarray(mask)
    in_maps = []
    for b in range(8):
        mb = np.where(mask[b], np.float32(-1e4), np.float32(0.0)).astype(np.float32)
        in_maps.append({
            "XT": bf(X[b].T),
            "YT8": f8(Y[b].T),
            "MB": mb,
            **shared,
        })
    return in_maps


def kernel(X, Y, mask, Wq, bq, Wk, bk, Wv, bv, Wo, bo, g1, b1, g2, b2,
           _trace=False):
    nc = _get_nc()
    in_maps = make_in_maps(X, Y, mask, Wq, bq, Wk, bk, Wv, bv, Wo, bo,
                           g1, b1, g2, b2)
    res = run_bass_kernel_spmd(nc, in_maps, core_ids=list(range(8)),
                               trace=_trace)
    out = np.stack([np.ascontiguousarray(res.results[b]["OT"].T)
                    for b in range(8)]).astype(np.float32)
    if _trace:
        return out, res
    return out


# revision 34
# speedup vs baseline: 1.0080x; 1.0080x over previous
"""Trainium2 Bass kernel for a masked-attention block (MAB).

Computation (per batch element):
    Q = X@Wq + bq ; K = Y@Wk + bk ; V = Y@Wv + bv
    logits = per-head Qh@Kh^T / 32, masked keys -> -inf, softmax over keys
    attn   = A @ Vh (concat heads)
    O1 = LN(Q + attn; g1,b1)
    O  = LN(O1 + relu(O1@Wo + bo); g2,b2)

Sharding: pure data-parallel, one batch element per NeuronCore (B=8 = 8 cores).

On-device dataflow is "feature-major": activations live in SBUF transposed
([model_dim -> 8x128 partitions, token -> free]).  With weights in natural
layout every matmul chains without any transposes.

Precision plan (correctness gate is rel_err < 2e-2, leaving large headroom):
  * Q-proj / logits / O-proj in bf16 (these touch the residual path directly,
    bf16 keeps their error ~0.5%).  bf16 moving operands are 1024 wide, which
    halves the matmul dispatch count vs fp32r's 512 limit.
  * K-proj / V-proj / softmax-exp / denominator / AV in fp8e4 with
    MatmulPerfMode.DoubleRow (2 contraction sub-tiles per pass = 2x MAC rate).
    All of these errors reach the output diluted ~30x: attention output is a
    ~900-key weighted mean added to the much larger Q residual, and the
    denominator/numerator share the same quantized exp so the softmax
    normalization error largely cancels.
  * LayerNorm stats via bf16 all-ones matmuls on bf16 activations
    (partition-dim reduction + broadcast in one shot).
  * V bias is folded into the attention epilogue (softmax weights sum to 1,
    so  sum_k A_k (V_k + bv) = pa*rc + bv), saving the bias-fold matmuls.

The host converts X->bf16, Y->fp8, Wq/Wo->bf16, Wk/Wv->fp8 and transposes
X/Y, which also cuts input HBM traffic from 28MB to ~9MB per core.
"""

import math
import numpy as np
from contextlib import ExitStack

import ml_dtypes

import concourse.bass as bass
import concourse.mybir as mybir
import concourse.tile as tile
from concourse import bacc
from concourse.bass_utils import run_bass_kernel_spmd

P = 128
NX = 1024
NY = 1024
DIM = 1024
H = 8
KO = DIM // P          # 8 partition sub-tiles of the model dim
QC = 512               # half-chunk of the query dim (for LN/O-proj pipelining)
NQC = NX // QC         # 2
F32 = mybir.dt.float32
BF16 = mybir.dt.bfloat16
F8 = mybir.dt.float8e4
AF = mybir.ActivationFunctionType
ALU = mybir.AluOpType
DR = mybir.MatmulPerfMode.DoubleRow
SCALE = 1.0 / 32.0     # 1/sqrt(DIM)
EPS = 1e-5
# walrus ships --enable-ldw-opt=false; flipping it to true dies in walrus
# codegen (visitInstLdweights INTERNAL_ERROR) with this kernel's DoubleRow
# matmuls, so it stays off.
ENABLE_LDW_OPT = False

_LDW_PATCHED = False


def _patch_ldw_opt():
    global _LDW_PATCHED
    if _LDW_PATCHED or not ENABLE_LDW_OPT:
        return
    import concourse.bass_utils as _bu
    _orig = _bu.run_command

    def _run_command(argv, **kwargs):
        argv = ["--enable-ldw-opt=true" if a == "--enable-ldw-opt=false" else a
                for a in argv]
        return _orig(argv, **kwargs)

    _bu.run_command = _run_command
    _LDW_PATCHED = True


def _build():
    _patch_ldw_opt()
    nc = bacc.Bacc("TRN2", target_bir_lowering=False, debug=False,
                   enable_asserts=False)

    # ---- DRAM I/O (per-core shapes) ----
    XT = nc.dram_tensor("XT", [DIM, NX], BF16, kind="ExternalInput").ap()
    YT8 = nc.dram_tensor("YT8", [DIM, NY], F8, kind="ExternalInput").ap()
    MB = nc.dram_tensor("MB", [NY], F32, kind="ExternalInput").ap()
    Wq = nc.dram_tensor("Wq", [DIM, DIM], BF16, kind="ExternalInput").ap()
    Wk = nc.dram_tensor("Wk", [DIM, DIM], F8, kind="ExternalInput").ap()
    Wv = nc.dram_tensor("Wv", [DIM, DIM], F8, kind="ExternalInput").ap()
    Wo = nc.dram_tensor("Wo", [DIM, DIM], BF16, kind="ExternalInput").ap()
    Vecs = {}
    for vname in ("bq", "bk", "bv", "bo", "g1", "b1", "g2", "b2"):
        Vecs[vname] = nc.dram_tensor(vname, [DIM], F32, kind="ExternalInput").ap()
    OT = nc.dram_tensor("OT", [DIM, NX], BF16, kind="ExternalOutput").ap()

    xt3 = XT.rearrange("(ko p) q -> p ko q", p=P)
    yt3 = YT8.rearrange("(ko p) q -> p ko q", p=P)
    wq3 = Wq.rearrange("(ko p) d -> p ko d", p=P)
    wk3 = Wk.rearrange("(ko p) d -> p ko d", p=P)
    wv3 = Wv.rearrange("(ko p) d -> p ko d", p=P)
    wo3 = Wo.rearrange("(ko p) d -> p ko d", p=P)
    ot3 = OT.rearrange("(do p) q -> p do q", p=P)

    with tile.TileContext(nc) as tc:
        with ExitStack() as octx:
            const = octx.enter_context(tc.tile_pool(name="const", bufs=1))
            # big activation tiles; o1t reuses qt's slot, z2t reuses zt's
            # (same tag + bufs=1 -> the tile framework rotates in place and
            # carries the WAR dependency for us)
            bigp = octx.enter_context(tc.tile_pool(name="big", bufs=1))

            # ---- constants ----
            ones_bf = const.tile([P, P], BF16, tag="onesbf", name="ones_bf")
            nc.vector.memset(ones_bf, 1.0)
            ones8 = const.tile([P, 2, P], F8, tag="ones8", name="ones8")
            nc.vector.memset(ones8, 1.0)
            eps_sb = const.tile([P, 1], F32, tag="eps", name="eps_sb")
            nc.vector.memset(eps_sb, EPS)

            # vec tiles are allocated here but their (scattered, slow) DMAs
            # are issued after the big input DMAs so they don't delay the
            # first matmuls
            def vec_pko(name):
                return const.tile([P, KO], F32, tag=f"v_{name}", name=f"{name}_sb")

            mb_sb = const.tile([P, KO], F32, tag="v_mb", name="mb_sb")
            bq_sb = vec_pko("bq")
            bk_sb = vec_pko("bk")
            bv_sb = vec_pko("bv")
            bo_sb = vec_pko("bo")
            g1_sb = vec_pko("g1")
            b1_sb = vec_pko("b1")
            g2_sb = vec_pko("g2")
            b2_sb = vec_pko("b2")

            def issue_vec_dmas():
                nc.sync.dma_start(mb_sb, MB.rearrange("(ko p) -> p ko", p=P))
                for name, t in (("bq", bq_sb), ("bk", bk_sb), ("bv", bv_sb),
                                ("bo", bo_sb), ("g1", g1_sb), ("b1", b1_sb),
                                ("g2", g2_sb), ("b2", b2_sb)):
                    nc.sync.dma_start(t, Vecs[name].rearrange("(ko p) -> p ko", p=P))

            # ---- big feature-major activation tiles ----
            qt = bigp.tile([P, KO, NX], BF16, tag="bigA", name="qt")
            ktm = bigp.tile([P, KO, NY], BF16, tag="bigK", name="ktm")
            vm8 = bigp.tile([P, KO, DIM], F8, tag="bigV", name="vm8")
            zt = bigp.tile([P, KO, NX], BF16, tag="bigB", name="zt")
            wo_sb = bigp.tile([P, KO, DIM], BF16, tag="bigW", name="wo_sb")

            # ============ Phase 1: projections (+ attention) ============
            with tc.tile_pool(name="io", bufs=1) as iop, \
                 tc.tile_pool(name="w1", bufs=1) as wp:
                yt8 = iop.tile([P, KO, NY], F8, tag="yt", name="yt8")
                xt = iop.tile([P, KO, NX], BF16, tag="xt", name="xt")
                wk8 = wp.tile([P, KO, DIM], F8, tag="wk", name="wk8")
                wq_sb = wp.tile([P, KO, DIM], BF16, tag="wq", name="wq_sb")
                wv8 = wp.tile([P, KO, DIM], F8, tag="wv", name="wv8")
                # DMA order == need order: K-proj inputs, Q-proj inputs,
                # V weights, O weights (arrives during attention)
                # interleave Y/Wk so the first K-proj k-pair (4 DMAs) lands
                # as early as possible -- the accumulation then streams
                # behind the DMA queue instead of waiting for all 16
                for k in range(KO):
                    nc.sync.dma_start(yt8[:, k, :], yt3[:, k, :])
                    nc.sync.dma_start(wk8[:, k, :], wk3[:, k, :])
                issue_vec_dmas()
                for k in range(KO):
                    nc.sync.dma_start(xt[:, k, :], xt3[:, k, :])
                for k in range(KO):
                    nc.sync.dma_start(wq_sb[:, k, :], wq3[:, k, :])
                for k in range(KO):
                    nc.sync.dma_start(wv8[:, k, :], wv3[:, k, :])
                for k in range(KO):
                    nc.sync.dma_start(wo_sb[:, k, :], wo3[:, k, :])

                with tc.tile_pool(name="kqp", bufs=2, space="PSUM") as kqp:
                    # ~36 dummy matmuls on constants while the first input
                    # DMAs stream in: keeps the PE busy so the HAM clock
                    # gate is at 8/8 (2.4 GHz) when the real matmuls start,
                    # instead of paying the cold 1.2 GHz ramp on them
                    warm_rhs = const.tile([P, QC], BF16, tag="warm",
                                          name="warm_rhs")
                    nc.vector.memset(warm_rhs, 0.0)
                    warm_ps = kqp.tile([P, QC], F32, tag="warmps",
                                       name="warm_ps")
                    for _ in range(36):
                        nc.tensor.matmul(warm_ps, lhsT=ones_bf, rhs=warm_rhs,
                                         start=True, stop=True)

                    # K-proj: fp8 DoubleRow, feature-major out -> ktm bf16
                    for no in range(KO):
                        for qc in range(NQC):
                            qs = slice(qc * QC, (qc + 1) * QC)
                            ps = kqp.tile([P, QC], F32, tag="kps",
                                          name=f"kps{no}{qc}")
                            for kp in range(KO // 2):
                                nc.tensor.matmul(
                                    ps,
                                    lhsT=wk8[:, 2 * kp:2 * kp + 2,
                                             no * P:(no + 1) * P],
                                    rhs=yt8[:, 2 * kp:2 * kp + 2, qs],
                                    start=(kp == 0), stop=(kp == KO // 2 - 1),
                                    perf_mode=DR)
                            nc.scalar.activation(
                                ktm[:, no, qs], ps, AF.Identity,
                                bias=bk_sb[:, no:no + 1], scale=1.0)

                    # Q-proj: bf16 -> qt bf16 (psum writes are 512-wide: a
                    # matmul output cannot cross a PSUM bank; the ACT reads
                    # the whole 2-bank tile in one op)
                    for no in range(KO):
                        psq = kqp.tile([P, NX], F32, tag="qps", name=f"qps{no}")
                        for k in range(KO):
                            for qc in range(NQC):
                                qs = slice(qc * QC, (qc + 1) * QC)
                                nc.tensor.matmul(
                                    psq[:, qs],
                                    lhsT=wq_sb[:, k, no * P:(no + 1) * P],
                                    rhs=xt[:, k, qs],
                                    start=(k == 0), stop=(k == KO - 1))
                        nc.scalar.activation(
                            qt[:, no, :], psq, AF.Identity,
                            bias=bq_sb[:, no:no + 1], scale=1.0)

                # ============ Phase 2: attention ============
                # V-proj matmuls are interleaved into the first heads: the
                # attention phase is ACT(exp)-bound, so the PE has idle slots.
                with tc.tile_pool(name="lgp", bufs=2, space="PSUM") as lgp, \
                     tc.tile_pool(name="vpp", bufs=2, space="PSUM") as vpp, \
                     tc.tile_pool(name="prp", bufs=1, space="PSUM") as prp, \
                     tc.tile_pool(name="pap", bufs=1, space="PSUM") as pap, \
                     tc.tile_pool(name="etp", bufs=2) as etp, \
                     tc.tile_pool(name="rcp", bufs=2) as rcp, \
                     tc.tile_pool(name="ept", bufs=3) as ept:

                    def vproj(ng):
                        # V in natural (token-major) layout, fp8 DoubleRow:
                        # V[y, n] = sum_k Y[y, k] Wv[k, n]
                        ns = slice(ng * QC, (ng + 1) * QC)
                        for yo in range(KO):
                            ps = vpp.tile([P, QC], F32, tag="vps",
                                          name=f"vps{ng}{yo}")
                            for kp in range(KO // 2):
                                nc.tensor.matmul(
                                    ps,
                                    lhsT=yt8[:, 2 * kp:2 * kp + 2,
                                             yo * P:(yo + 1) * P],
                                    rhs=wv8[:, 2 * kp:2 * kp + 2, ns],
                                    start=(kp == 0), stop=(kp == KO // 2 - 1),
                                    perf_mode=DR)
                            # no bias here: bv is folded into the epilogue
                            nc.vector.tensor_copy(vm8[:, yo, ns], ps)

                    def logits_exp(h):
                        # logitsT[k, q] = sum_d KT_h[d, k] QT_h[d, q]; exp with
                        # mask bias per key (partition) and 1/32 scale, output
                        # straight to fp8 (feeds only the fp8 DR matmuls).
                        et8 = etp.tile([P, KO, NY], F8, tag="et", name=f"et{h}")
                        for kt in range(KO):
                            pl = lgp.tile([P, NX], F32, tag="lg",
                                          name=f"pl{h}{kt}")
                            for qc in range(NQC):
                                qs = slice(qc * QC, (qc + 1) * QC)
                                nc.tensor.matmul(
                                    pl[:, qs],
                                    lhsT=ktm[:, h, kt * P:(kt + 1) * P],
                                    rhs=qt[:, h, qs],
                                    start=True, stop=True)
                            nc.scalar.activation(
                                et8[:, kt, :], pl, AF.Exp,
                                bias=mb_sb[:, kt:kt + 1], scale=SCALE)
                        return et8

                    def denom_av(h, et8):
                        # softmax denominator via all-ones fp8 DR matmul
                        # (partition reduction AND broadcast in one shot),
                        # then attnT_h = V^T expT, normalized + bv + Q resid.
                        rc = rcp.tile([P, NX], F32, tag="rc", name=f"rc{h}")
                        for qc in range(NQC):
                            qs = slice(qc * QC, (qc + 1) * QC)
                            pr = prp.tile([P, QC], F32, tag="pr",
                                          name=f"pr{h}{qc}")
                            for kp in range(KO // 2):
                                nc.tensor.matmul(
                                    pr, lhsT=ones8,
                                    rhs=et8[:, 2 * kp:2 * kp + 2, qs],
                                    start=(kp == 0), stop=(kp == KO // 2 - 1),
                                    perf_mode=DR)
                            nc.vector.reciprocal_approx_fast(rc[:, qs], pr)
                        for qc in range(NQC):
                            qs = slice(qc * QC, (qc + 1) * QC)
                            pa = pap.tile([P, QC], F32, tag="pa",
                                          name=f"pa{h}{qc}")
                            for kp in range(KO // 2):
                                nc.tensor.matmul(
                                    pa,
                                    lhsT=vm8[:, 2 * kp:2 * kp + 2,
                                             h * P:(h + 1) * P],
                                    rhs=et8[:, 2 * kp:2 * kp + 2, qs],
                                    start=(kp == 0), stop=(kp == KO // 2 - 1),
                                    perf_mode=DR)
                            t1 = ept.tile([P, QC], BF16, tag="t1",
                                          name=f"t1{h}{qc}")
                            nc.vector.tensor_mul(t1, pa, rc[:, qs])
                            # zt = (pa*rc + bv) + qt   (bv fold: sum_k A_k = 1)
                            nc.vector.scalar_tensor_tensor(
                                zt[:, h, qs], t1, bv_sb[:, h:h + 1],
                                qt[:, h, qs], op0=ALU.add, op1=ALU.add)

                    # software pipeline: head h's logits/exp (PE+ACT) run while
                    # head h-1's denominator+AV (PE) wait on h-1's exp
                    prev = None
                    for h in range(H):
                        et8 = logits_exp(h)
                        if h == 0:
                            vproj(0)
                        elif h == 1:
                            vproj(1)
                        if prev is not None:
                            denom_av(h - 1, prev)
                        prev = et8
                    denom_av(H - 1, prev)

            # ---- LayerNorm over the model dim (partition direction) ----
            # engine split: squares + final scale/shift on ACT, stats chain +
            # normalize on DVE, reductions on PE (the back half is otherwise
            # DVE-serialized)
            def layernorm(in_sb, q0, QL, sqp, stp, spp, emit_out,
                          use_gpsimd=True):
                qc = q0 // QL
                qs = slice(q0, q0 + QL)
                # stats psums allocated at full chunk size and sliced, so
                # 256-wide chunks don't cost extra PSUM bank tags
                pmu = spp.tile([P, QC], F32, tag="pmu", name=f"pmu{qc}")[:, :QL]
                ps2 = spp.tile([P, QC], F32, tag="ps2", name=f"ps2{qc}")[:, :QL]
                for do in range(KO):
                    nc.tensor.matmul(pmu, lhsT=ones_bf,
                                     rhs=in_sb[:, do, qs],
                                     start=(do == 0), stop=(do == KO - 1))
                # squared sums: ACT writes fp8 pairs, reduced with a
                # DoubleRow all-ones matmul (half the matmul count; the fp8
                # quantization of x^2 averages out over the 1024 features)
                for dp in range(KO // 2):
                    sq = sqp.tile([P, 2, QL], F8, tag=f"sq{QL}", name=f"sq{qc}{dp}")
                    nc.scalar.square(sq[:, 0, :], in_sb[:, 2 * dp, qs])
                    nc.scalar.square(sq[:, 1, :], in_sb[:, 2 * dp + 1, qs])
                    nc.tensor.matmul(ps2, lhsT=ones8, rhs=sq,
                                     start=(dp == 0), stop=(dp == KO // 2 - 1),
                                     perf_mode=DR)
                mu = stp.tile([P, QL], F32, tag=f"mu{QL}", name=f"mu{qc}")
                nc.vector.tensor_scalar_mul(mu, pmu, 1.0 / DIM)
                msq = stp.tile([P, QL], F32, tag=f"msq{QL}", name=f"msq{qc}")
                nc.vector.tensor_mul(msq, mu, mu)
                sd = stp.tile([P, QL], F32, tag=f"sd{QL}", name=f"sd{qc}")
                nc.vector.scalar_tensor_tensor(
                    sd, ps2, 1.0 / DIM, msq,
                    op0=ALU.mult, op1=ALU.subtract)
                nc.scalar.activation(sd, sd, AF.Sqrt, bias=eps_sb, scale=1.0)
                # bf16 rsig/mrs/t: both normalize tensor-tensor ops then run
                # in the DVE 2x_1P packed mode (16-bit, unit stride); the
                # last two feature blocks go to the otherwise-idle GpSimd
                rsig32 = stp.tile([P, QL], F32, tag=f"rsig32{QL}", name=f"rsig32{qc}")
                nc.vector.reciprocal_approx_fast(rsig32, sd)
                rsig = stp.tile([P, QL], BF16, tag=f"rsig{QL}", name=f"rsig{qc}")
                nc.vector.tensor_copy(rsig, rsig32)
                mrs = stp.tile([P, QL], BF16, tag=f"mrs{QL}", name=f"mrs{qc}")
                nc.vector.tensor_mul(mrs, mu, rsig32)
                for do in range(KO):
                    # GpSimd runs ~4x slower than packed DVE: give it one
                    # block mid-kernel, none in LN2 where its latency would
                    # sit on the critical end-of-kernel path
                    eng = nc.gpsimd if (use_gpsimd and do == KO - 1) else nc.vector
                    t = sqp.tile([P, QL], BF16, tag=f"t{QL}", name=f"t{qc}{do}")
                    eng.tensor_mul(t, in_sb[:, do, qs], rsig)
                    eng.tensor_sub(t, t, mrs)
                    emit_out(do, qs, t)

            # ============ Phase 3: LN1 / O-proj / LN2, pipelined by
            # query half-chunk so LN's DVE latency hides under O-proj ====
            o1t = bigp.tile([P, KO, NX], BF16, tag="bigA", name="o1t")
            z2t = bigp.tile([P, KO, NX], BF16, tag="bigB", name="z2t")

            def emit_o1(do, qs, t):
                # o1 = t*g1 + b1 on ACT (activation computes f(in*scale+bias))
                nc.scalar.activation(
                    o1t[:, do, qs], t, AF.Identity,
                    bias=b1_sb[:, do:do + 1], scale=g1_sb[:, do:do + 1])

            with tc.tile_pool(name="sq1", bufs=3) as sqp1, \
                 tc.tile_pool(name="st1", bufs=2) as stp1, \
                 tc.tile_pool(name="sq2", bufs=4) as sqp2, \
                 tc.tile_pool(name="st2", bufs=2) as stp2, \
                 tc.tile_pool(name="out", bufs=4) as outp, \
                 tc.tile_pool(name="sp1", bufs=1, space="PSUM") as spp1, \
                 tc.tile_pool(name="gp3", bufs=4, space="PSUM") as pp3, \
                 tc.tile_pool(name="sp2", bufs=1, space="PSUM") as spp2:

                def oproj(q0, ql):
                    # HT[n, q] = sum_d Wo[d, n] O1T[d, q]; z2 = o1 + relu(H+bo)
                    # NOTE: keep no-outer/k-inner -- a k-outer variant that
                    # cycles psum banks every matmul measured 15-20% slower
                    # (HAM psum-queue depth-cycling)
                    qs = slice(q0, q0 + ql)
                    qc = q0 // 256
                    for no in range(KO):
                        ps = pp3.tile([P, QC], F32, tag="ps",
                                      name=f"ps_o{qc}{no}")[:, :ql]
                        for k in range(KO):
                            nc.tensor.matmul(
                                ps,
                                lhsT=wo_sb[:, k, no * P:(no + 1) * P],
                                rhs=o1t[:, k, qs],
                                start=(k == 0), stop=(k == KO - 1))
                        ht = sqp2.tile([P, QC], BF16, tag="ht",
                                       name=f"ht{qc}{no}")[:, :ql]
                        nc.scalar.activation(ht, ps, AF.Relu,
                                             bias=bo_sb[:, no:no + 1], scale=1.0)
                        # all-bf16 add -> DVE 2x packed mode
                        nc.vector.tensor_add(z2t[:, no, qs], ht, o1t[:, no, qs])

                def emit_o2(do, qs, t):
                    ql = qs.stop - qs.start
                    o = outp.tile([P, ql], BF16, tag=f"o{ql}", name=f"o{do}")
                    nc.scalar.activation(
                        o, t, AF.Identity,
                        bias=b2_sb[:, do:do + 1], scale=g2_sb[:, do:do + 1])
                    nc.sync.dma_start(ot3[:, do, qs], o)

                layernorm(zt, 0, QC, sqp1, stp1, spp1, emit_o1)
                oproj(0, QC)
                layernorm(zt, QC, QC, sqp1, stp1, spp1, emit_o1)
                oproj(QC, QC)
                layernorm(z2t, 0, QC, sqp2, stp2, spp2, emit_o2)
                layernorm(z2t, QC, QC, sqp2, stp2, spp2, emit_o2,
                          use_gpsimd=False)

    nc.compile()
    return nc


_CACHE = {}


def _get_nc():
    if "nc" not in _CACHE:
        _CACHE["nc"] = _build()
    return _CACHE["nc"]


def make_in_maps(X, Y, mask, Wq, bq, Wk, bk, Wv, bv, Wo, bo, g1, b1, g2, b2):
    f32 = lambda a: np.ascontiguousarray(np.asarray(a, dtype=np.float32))
    bf = lambda a: np.ascontiguousarray(
        np.asarray(a, dtype=np.float32).astype(ml_dtypes.bfloat16))
    f8 = lambda a: np.ascontiguousarray(
        np.asarray(a, dtype=np.float32).astype(ml_dtypes.float8_e4m3))
    shared = {
        "Wq": bf(Wq), "Wk": f8(Wk), "Wv": f8(Wv), "Wo": bf(Wo),
        "bq": f32(bq), "bk": f32(bk), "bv": f32(bv), "bo": f32(bo),
        "g1": f32(g1), "b1": f32(b1), "g2": f32(g2), "b2": f32(b2),
    }
    X = np.asarray(X, dtype=np.float32)
    Y = np.asarray(Y, dtype=np.float32)
    mask = np.asarray(mask)
    in_maps = []
    for b in range(8):
        mb = np.where(mask[b], np.float32(-1e4), np.float32(0.0)).astype(np.float32)
        in_maps.append({
            "XT": bf(X[b].T),
            "YT8": f8(Y[b].T),
            "MB": mb,
            **shared,
        })
    return in_maps


def kernel(X, Y, mask, Wq, bq, Wk, bk, Wv, bv, Wo, bo, g1, b1, g2, b2,
           _trace=False):
    nc = _get_nc()
    in_maps = make_in_maps(X, Y, mask, Wq, bq, Wk, bk, Wv, bv, Wo, bo,
                           g1, b1, g2, b2)
    res = run_bass_kernel_spmd(nc, in_maps, core_ids=list(range(8)),
                               trace=_trace)
    out = np.stack([np.ascontiguousarray(res.results[b]["OT"].T)
                    for b in range(8)]).astype(np.float32)
    if _trace:
        return out, res
    return out
